# revision 8
# baseline (speedup 1.0000x reference)
"""Dense->CSR converter on 8 Trainium2 NeuronCores (Bass/Tile).

Contract: kernel(x) takes the FULL x [4096, 8192] fp32 and returns the
same tuple as the reference: (crow_indices [4097] i32,
col_indices_padded [numel] i32, values_padded [numel] f32, nnz i32).

Sharding: row-parallel — core c gets rows [512c, 512c+512).  Each core
emits (a) per-run nonzero counts and (b) its fully compacted
(col, value) stream, built on-device.  The host computes crow from the
counts and concatenates the 8 per-core streams (pure unshard/stitch).

Device algorithm per core (8 tiles of [128 partitions x 4096]):
  mask=(x!=0) -> per-partition cumsum (DVE scan) -> scatter destinations
  -> three per-partition 16-bit local_scatter ops (GPSIMD) compact the
  column ids and the hi/lo halves of the fp32 values -> PE matmuls with
  a run-order comparison matrix turn per-run counts into exact element
  offsets -> indirect DMAs with compute-op ADD write each run's
  zero-padded fixed-size segment at its exact offset into the
  (pre-zeroed, donated) DRAM stream buffers.

Because segment windows [O_r, O_r+Lc) overlap only the NEXT run's
region (two consecutive run counts always exceed Lc), the scatter is
split into two phases: even-rank runs first, odd-rank runs after them.
Within a phase all windows are disjoint, and the zero padding of a
window adds 0 onto the neighbour's already-final data — so descriptor
order inside a phase never matters.  Runs are permuted across
partitions (even ranks in partitions 0..63, odd in 64..127) so each
phase is a contiguous partition block.
"""

from contextlib import ExitStack

import numpy as np

import concourse.tile as tile
from concourse import bacc, mybir
from concourse.bass import IndirectOffsetOnAxis
from concourse.bass_utils import run_bass_kernel_spmd
from concourse.tile import add_dep_helper

F32 = mybir.dt.float32
I16 = mybir.dt.int16
I32 = mybir.dt.int32

ROWS, COLS = 4096, 8192
NCORES = 8
R = ROWS // NCORES          # rows per core
P = 128                     # partitions
L = 4096                    # flat run length per partition (half a row)
Lc = 1536                   # per-run compacted capacity (mean 1229, +10 sigma)
T = (R * COLS) // (P * L)   # tiles per core
CAP = 1441792               # per-core stream capacity (mean nnz 1.26M)
NUMEL = ROWS * COLS
HALF = P // 2


def _run_rank():
    """Partition p holds global run (tile*P + rank[p])."""
    p = np.arange(P)
    return 2 * (p % HALF) + p // HALF


def _make_consts():
    rank = _run_rank()
    # strict "before in run order": M[q, p] = 1 iff rank[q] < rank[p]
    triu = (rank[:, None] < rank[None, :]).astype(np.float32)
    # col index of element j of the run held by partition p
    par = rank % 2
    j = np.arange(L)[None, :]
    colvals = (j + (COLS // 2) * par[:, None]).astype(np.int16)
    return triu, colvals


BIGOFF = 1 << 30


def _emit_odd(nc, vals_d, cols_d, pending, next_even_insts):
    o_odd, vals32, cols32, even_insts = pending
    for dram, src in ((vals_d, vals32), (cols_d, cols32)):
        di = nc.gpsimd.indirect_dma_start(
            out=dram[:].rearrange("(a b) -> a b", b=1),
            out_offset=IndirectOffsetOnAxis(ap=o_odd[:, :1], axis=0),
            in_=src[:],
            in_offset=None,
            compute_op=mybir.AluOpType.add,
            bounds_check=CAP - 1,
            oob_is_err=False,
        )
        for ei in even_insts + next_even_insts:
            add_dep_helper(di.ins, ei.ins, sync=True,
                           reason="odd-phase adds after even-phase writes")


def _build(ctx: ExitStack, tc: tile.TileContext, outs, ins):
    nc = tc.nc
    vals_d, cols_d, cnts_d = outs
    x_d, triu_d, colv_d = ins

    x_runs = x_d.rearrange("r (h f) -> (r h) f", f=L)
    # partition (par*HALF + q) <- run (t*P + 2q + par)
    x_tiles = x_runs.rearrange("(t q par) f -> t par q f", par=2, q=HALF)

    io = ctx.enter_context(tc.tile_pool(name="io", bufs=2))
    consts = ctx.enter_context(tc.tile_pool(name="consts", bufs=1))
    ephem = ctx.enter_context(tc.tile_pool(name="ephem", bufs=1))
    ephem2 = ctx.enter_context(tc.tile_pool(name="ephem2", bufs=2))
    work = ctx.enter_context(tc.tile_pool(name="work", bufs=3))
    small = ctx.enter_context(tc.tile_pool(name="small", bufs=1))
    sm2 = ctx.enter_context(tc.tile_pool(name="sm2", bufs=2))
    psum = ctx.enter_context(tc.tile_pool(name="psum", bufs=1, space="PSUM"))

    triu = consts.tile([P, P], F32)
    nc.sync.dma_start(triu[:], triu_d[:])
    colv = consts.tile([P, L], I16)
    nc.sync.dma_start(colv[:], colv_d[:])
    ones_col = consts.tile([P, 1], F32)
    nc.vector.memset(ones_col[:], 1.0)
    ones_row = consts.tile([1, P], F32)
    nc.vector.memset(ones_row[:], 1.0)

    cnts_all = small.tile([P, T], F32)
    carry_sb = small.tile([1, 1], F32)
    nc.vector.memset(carry_sb[:], 0.0)
    pending = None

    for t in range(T):
        xt = io.tile([P, L], F32, tag="xt")
        nc.sync.dma_start(xt[:], x_tiles[t])

        # chunked chain: mask/scan/stt/cast in two halves so the ACT cast
        # of half 0 overlaps the DVE work of half 1 (scan chains via
        # initial=), shortening the latency to the first scatter
        mask = ephem.tile([P, L], F32, tag="mask")
        counts_t = sm2.tile([P, 1], F32, tag="counts")
        cnt = ephem.tile([P, L], F32, tag="cnt")
        dest16 = ephem2.tile([P, L], I16, tag="dest16")
        H2 = L // 2
        half_carry = sm2.tile([P, 1], F32, tag="halfcarry")
        for h in (0, 1):
            sl = slice(h * H2, (h + 1) * H2)
            nc.vector.tensor_scalar(mask[:, sl], xt[:, sl], 0.0, 0.0,
                                    mybir.AluOpType.not_equal,
                                    mybir.AluOpType.add)
            init = 0.0 if h == 0 else half_carry[:, :1]
            nc.vector.tensor_tensor_scan(cnt[:, sl], mask[:, sl],
                                         mask[:, sl], init,
                                         mybir.AluOpType.add,
                                         mybir.AluOpType.bypass)
            # snapshot the raw running count before the in-place stt
            if h == 0:
                nc.vector.tensor_copy(half_carry[:], cnt[:, H2 - 1:H2])
            else:
                nc.vector.tensor_copy(counts_t[:], cnt[:, L - 1:L])
                nc.vector.tensor_copy(cnts_all[:, t:t + 1], counts_t[:])
            nc.vector.scalar_tensor_tensor(cnt[:, sl], mask[:, sl],
                                           float(L), cnt[:, sl],
                                           mybir.AluOpType.mult,
                                           mybir.AluOpType.add)
            nc.scalar.activation(dest16[:, sl], cnt[:, sl],
                                 mybir.ActivationFunctionType.Copy,
                                 bias=-float(L + 1), scale=1.0)

        x16 = xt[:].bitcast(I16).rearrange("p (f two) -> p f two", two=2)
        lo16 = ephem2.tile([P, L], I16, tag="lo16")
        hi16 = ephem2.tile([P, L], I16, tag="hi16")
        nc.scalar.copy(lo16[:], x16[:, :, 0])
        nc.vector.tensor_copy(hi16[:], x16[:, :, 1])

        comp16 = work.tile([P, Lc], I16, tag="comp16")
        vhi = work.tile([P, Lc], I16, tag="vhi")
        vlo = work.tile([P, Lc], I16, tag="vlo")
        nc.gpsimd.local_scatter(comp16[:], colv[:], dest16[:], P, Lc, L)
        nc.gpsimd.local_scatter(vhi[:], hi16[:], dest16[:], P, Lc, L)
        nc.gpsimd.local_scatter(vlo[:], lo16[:], dest16[:], P, Lc, L)

        vals32 = work.tile([P, Lc], F32, tag="vals32")
        v16 = vals32[:].bitcast(I16).rearrange("p (f two) -> p f two", two=2)
        nc.scalar.copy(v16[:, :, 0], vlo[:])
        nc.vector.tensor_copy(v16[:, :, 1], vhi[:])

        cols32 = work.tile([P, Lc], F32, tag="cols32")
        nc.scalar.copy(cols32[:], comp16[:])

        o_ps = psum.tile([P, 1], F32, tag=f"ops{t % 2}")
        nc.tensor.matmul(o_ps[:], lhsT=triu[:], rhs=counts_t[:],
                         start=True, stop=False)
        nc.tensor.matmul(o_ps[:], lhsT=ones_row[:], rhs=carry_sb[:],
                         start=False, stop=True)
        tot_ps = psum.tile([1, 1], F32, tag=f"tot{t % 2}")
        nc.tensor.matmul(tot_ps[:], lhsT=counts_t[:], rhs=ones_col[:],
                         start=True, stop=True)
        nc.vector.tensor_tensor(carry_sb[:], carry_sb[:], tot_ps[:],
                                mybir.AluOpType.add)

        # Full 128-descriptor DMAs; the inactive half's offsets point past
        # bounds_check so those descriptors are skipped (oob_is_err=False).
        o_even = sm2.tile([P, 1], I32, tag="oeven")
        nc.vector.tensor_copy(o_even[:], o_ps[:])
        nc.vector.memset(o_even[HALF:, :], BIGOFF)
        o_odd = sm2.tile([P, 1], I32, tag="oodd")
        nc.scalar.copy(o_odd[:], o_ps[:])
        nc.vector.memset(o_odd[:HALF, :], BIGOFF)

        even_insts = []
        for dram, src in ((vals_d, vals32), (cols_d, cols32)):
            di = nc.gpsimd.indirect_dma_start(
                out=dram[:].rearrange("(a b) -> a b", b=1),
                out_offset=IndirectOffsetOnAxis(ap=o_even[:, :1], axis=0),
                in_=src[:],
                in_offset=None,
                compute_op=mybir.AluOpType.add,
                bounds_check=CAP - 1,
                oob_is_err=False,
            )
            even_insts.append(di)

        if pending is not None:
            _emit_odd(nc, vals_d, cols_d, pending, even_insts)
        pending = (o_odd, vals32, cols32, even_insts)

    _emit_odd(nc, vals_d, cols_d, pending, [])
    nc.sync.dma_start(cnts_d[:], cnts_all[:])


_CACHED = None


def _get_kernel():
    global _CACHED
    if _CACHED is not None:
        return _CACHED
    nc = bacc.Bacc("TRN2", target_bir_lowering=False, debug=False)
    x_d = nc.dram_tensor("x", [R, COLS], F32, kind="ExternalInput")
    triu_d = nc.dram_tensor("triu", [P, P], F32, kind="ExternalInput")
    colv_d = nc.dram_tensor("colvals", [P, L], I16, kind="ExternalInput")
    vals_d = nc.dram_tensor("vals", [CAP], F32, kind="ExternalOutput")
    cols_d = nc.dram_tensor("cols", [CAP], F32, kind="ExternalOutput")
    cnts_d = nc.dram_tensor("cnts", [P, T], F32, kind="ExternalOutput")
    with tile.TileContext(nc) as tc:
        with ExitStack() as ctx:
            _build(ctx, tc,
                   (vals_d.ap(), cols_d.ap(), cnts_d.ap()),
                   (x_d.ap(), triu_d.ap(), colv_d.ap()))
    nc.compile()
    _CACHED = nc
    return nc


def _run_device(x, trace=False, **kw):
    nc = _get_kernel()
    triu, colvals = _make_consts()
    x = np.ascontiguousarray(np.asarray(x, dtype=np.float32))
    in_maps = [
        {"x": x[c * R:(c + 1) * R], "triu": triu, "colvals": colvals}
        for c in range(NCORES)
    ]
    return run_bass_kernel_spmd(nc, in_maps, list(range(NCORES)),
                                trace=trace, **kw)


def kernel(x):
    x = np.ascontiguousarray(np.asarray(x, dtype=np.float32))
    res = _run_device(x)
    return assemble(res.results, x)


def assemble(results, x):
    rank = _run_rank()
    inv = np.argsort(rank)
    crow = np.zeros(ROWS + 1, np.int64)
    col_indices = np.full(NUMEL, -1, np.int32)
    values = np.zeros(NUMEL, np.float32)
    pos = 0
    row_counts_all = []
    for c in range(NCORES):
        r = results[c]
        cnts = r["cnts"]                       # [P, T]; cnts[p,t] = run t*P+rank[p]
        counts_runs = np.rint(cnts[inv, :].T).astype(np.int64).reshape(-1)
        assert counts_runs.max() < Lc, "per-run capacity exceeded"
        n = int(counts_runs.sum())
        assert n + Lc <= CAP, "per-core stream capacity exceeded"
        pairs = counts_runs[:-1] + counts_runs[1:]
        assert pairs.min() >= Lc, "phase-disjointness violated"
        row_counts_all.append(counts_runs.reshape(R, 2).sum(1))
        values[pos:pos + n] = r["vals"][:n]
        col_indices[pos:pos + n] = np.rint(r["cols"][:n]).astype(np.int32)
        pos += n
    row_counts = np.concatenate(row_counts_all)
    crow[1:] = np.cumsum(row_counts)
    crow = crow.astype(np.int32)
    nnz = np.int32(pos)

    # Replicate the reference's col_ids quirk: jax lowers
    # `arange(numel, int32) % COLS` through f32 on both the neuron and
    # cpu backends, which yields -1 (instead of COLS-1) for flat index
    # 8192*r + 8191 once it exceeds 2^23, i.e. for every row r >= 1024
    # whose last column holds a nonzero.  That element is the last entry
    # of row r's stream segment.
    rows = np.flatnonzero(x[:, COLS - 1] != 0)
    rows = rows[rows * COLS + (COLS - 1) >= 1 << 23]
    col_indices[crow[rows + 1].astype(np.int64) - 1] = -1
    return crow, col_indices, values, nnz
